# revision 16
# baseline (speedup 1.0000x reference)
"""Causal self-attention (B=4, S=2048, E=1024, H=16) on 8 Trainium2 cores.

Sharding: core c = (batch b = c//2, head-group g = c%2). Each core computes
batch b with 8 heads (512 of the 1024 embed dims): QKV column-sharded,
per-head causal attention, output projection row-sharded. Host sums the two
partial outputs per batch (no on-device collectives needed).

Device kernel layout notes:
  - x is passed host-transposed as xT [E, S]; all projections consume it as
    the matmul operand with E on the partition (contraction) dim.
  - qT/kT are produced directly transposed [d, s] (d = head dims, 2 heads
    per 128-partition block).
  - scores are computed transposed: scoresT[k, q] = sum_d kT[d,k] qT[d,q],
    so the attn @ V matmul can contract k on the partition dim.
  - causal mask: strictly-lower K-tiles need no mask; diagonal-region tiles
    get a -3e10 additive mask injected into PSUM via an identity matmul.
  - softmax denominator: V tiles are augmented with a ones column, so the
    attn @ V_aug matmul emits the row sum in output row 64 for free.
    1/r = Exp(-Ln(r)) on ScalarE (one ACT table set for everything),
    broadcast across partitions on GPSIMD, multiplied on VectorE.
  - all big matmuls run as float32r (1 cycle/row at N>=256; plain fp32 is
    4 cycles/row).
"""

import os
import sys

import numpy as np

for _p in ("/opt/trn_rl_repo", "/root/.axon_site/_ro/trn_rl_repo"):
    if os.path.isdir(_p) and _p not in sys.path:
        sys.path.insert(0, _p)

import concourse.bass as bass
import concourse.bacc as bacc
import concourse.mybir as mybir
import concourse.tile as tile
from concourse import bass_utils, library_config

EMBED = 1024
NHEADS = 16
HDIM = 64
BATCH = 4
SEQ = 2048
NCORES = 8

DBLKS = 4   # 128-wide d blocks per core (2 heads each)
EBLKS = 8   # 128-wide embed blocks
SCH = 4     # 512-wide s chunks
STL = 16    # 128-wide s tiles
NEG = -3.0e10
SCALE = 1.0 / 8.0  # 1/sqrt(HDIM)

F32 = mybir.dt.float32
F32R = mybir.dt.float32r
AF = mybir.ActivationFunctionType
ALU = mybir.AluOpType

LAST_EXEC_NS = None


def _r(ap):
    return ap.bitcast(F32R)


def build_bass():
    """Build the single-core SPMD Bass program (same program, 8 data shards)."""
    nc = bacc.Bacc(trn_type="TRN2", target_bir_lowering=False, debug=False)

    xT = nc.dram_tensor("xT", [EBLKS, 128, SEQ], F32R, kind="ExternalInput").ap()
    wq = nc.dram_tensor("wq", [EBLKS, DBLKS, 128, 128], F32R, kind="ExternalInput").ap()
    wk = nc.dram_tensor("wk", [EBLKS, DBLKS, 128, 128], F32R, kind="ExternalInput").ap()
    wv = nc.dram_tensor("wv", [EBLKS, 2, 128, 256], F32R, kind="ExternalInput").ap()
    wo = nc.dram_tensor("wo", [DBLKS, EBLKS, 128, 128], F32R, kind="ExternalInput").ap()
    bqd = nc.dram_tensor("bqd", [128, DBLKS], F32, kind="ExternalInput").ap()
    bkd = nc.dram_tensor("bkd", [128, DBLKS], F32, kind="ExternalInput").ap()
    bvb = nc.dram_tensor("bvb", [2, 128, 256], F32, kind="ExternalInput").ap()
    msk = nc.dram_tensor("msk", [4, 128, 512], F32R, kind="ExternalInput").ap()
    iden = nc.dram_tensor("iden", [128, 128], F32R, kind="ExternalInput").ap()
    iden64 = nc.dram_tensor("iden64", [64, 64], F32, kind="ExternalInput").ap()
    ones64 = nc.dram_tensor("ones64", [1, 64], F32R, kind="ExternalInput").ap()
    onescol = nc.dram_tensor("onescol", [128, 4], F32R, kind="ExternalInput").ap()
    out = nc.dram_tensor("out", [EBLKS, 128, SEQ], F32, kind="ExternalOutput").ap()

    with tile.TileContext(nc) as tc:
        _emit(tc, xT, wq, wk, wv, wo, bqd, bkd, bvb, msk, iden, iden64, ones64, onescol, out)
    nc.compile()
    return nc


def _emit(tc, xT, wq, wk, wv, wo, bqd, bkd, bvb, msk, iden, iden64, ones64, onescol, out):
    from contextlib import ExitStack

    nc = tc.nc
    ctx = ExitStack()
    with ctx:
        const = ctx.enter_context(tc.tile_pool(name="const", bufs=1))
        # PSUM: pmisc outlives the attention-phase pools (shift + Wo output).
        pmisc = ctx.enter_context(tc.tile_pool(name="pmisc", bufs=2, space="PSUM"))

        xt_pool = ctx.enter_context(tc.tile_pool(name="xt", bufs=10))
        w_pool = ctx.enter_context(tc.tile_pool(name="w", bufs=1))
        qk_pool = ctx.enter_context(tc.tile_pool(name="qk", bufs=2))
        v_pool = ctx.enter_context(tc.tile_pool(name="v", bufs=32))
        att_pool = ctx.enter_context(tc.tile_pool(name="att", bufs=3))
        ot_pool = ctx.enter_context(tc.tile_pool(name="ot", bufs=4))
        s_pool = ctx.enter_context(tc.tile_pool(name="s", bufs=2))
        y_pool = ctx.enter_context(tc.tile_pool(name="y", bufs=3))

        # ---- constants ----
        m_sb = []
        for rr in range(4):
            m_t = const.tile([128, 512], F32R, name=f"mask{rr}", tag=f"mask{rr}")
            nc.sync.dma_start(out=m_t, in_=msk[rr])
            m_sb.append(m_t)
        id_sb = const.tile([128, 128], F32R, name="id_sb")
        nc.sync.dma_start(out=id_sb, in_=iden)
        id64_sb = const.tile([64, 64], F32, name="id64_sb")
        nc.sync.dma_start(out=id64_sb, in_=iden64)
        ones_sb = const.tile([1, 64], F32R, name="ones_sb")
        nc.sync.dma_start(out=ones_sb, in_=ones64)
        bq_sb = const.tile([128, DBLKS], F32, name="bq_sb")
        nc.sync.dma_start(out=bq_sb, in_=bqd)
        bk_sb = const.tile([128, DBLKS], F32, name="bk_sb")
        nc.sync.dma_start(out=bk_sb, in_=bkd)
        bvb_sb = const.tile([128, 2, 256], F32, name="bvb_sb")
        nc.sync.dma_start(out=bvb_sb, in_=bvb.rearrange("v p c -> p v c"))
        wo_sb = const.tile([128, DBLKS, EBLKS, 128], F32R, name="wo_sb")
        nc.sync.dma_start(out=wo_sb, in_=wo.rearrange("d e p c -> p d e c"))

        oT = []  # per-dblk combined [128 d, SEQ] normalized attention output
        vaug_pairs = [None, None]  # per pair: list of 16 [128, 4, 65] tiles

        with (
            tc.tile_pool(name="pq", bufs=2, space="PSUM") as pq,
            tc.tile_pool(name="psc", bufs=2, space="PSUM") as psc,
            tc.tile_pool(name="pot", bufs=2, space="PSUM") as pot,
        ):
            for d in range(DBLKS):
                pair, side = divmod(d, 2)

                wq_t = []
                wk_t = []
                for e in range(EBLKS):
                    wq_te = w_pool.tile(
                        [128, 128], F32R, name=f"wq_{d}_{e}", tag="wqk", bufs=18
                    )
                    nc.sync.dma_start(out=wq_te, in_=wq[e, d])
                    wq_t.append(wq_te)
                    wk_te = w_pool.tile(
                        [128, 128], F32R, name=f"wk_{d}_{e}", tag="wqk", bufs=18
                    )
                    nc.sync.dma_start(out=wk_te, in_=wk[e, d])
                    wk_t.append(wk_te)
                wv_t = []
                if side == 0:
                    for e in range(EBLKS):
                        wv_te = w_pool.tile(
                            [128, 256], F32R, name=f"wv_{pair}_{e}", tag="wv", bufs=9
                        )
                        nc.sync.dma_start(out=wv_te, in_=wv[e, pair])
                        wv_t.append(wv_te)
                    vaug_pairs[pair] = [None] * STL

                qT = qk_pool.tile([128, SEQ], F32R, name=f"qT_{d}", tag="qT")
                kT = qk_pool.tile([128, SEQ], F32R, name=f"kT_{d}", tag="kT")

                # ---- projections for this d block ----
                for sc in range(SCH):
                    cols = slice(sc * 512, (sc + 1) * 512)
                    xts = []
                    for e in range(EBLKS):
                        x_te = xt_pool.tile(
                            [128, 512], F32R, name=f"x_{d}_{sc}_{e}", tag="xt"
                        )
                        nc.sync.dma_start(out=x_te, in_=xT[e, :, cols])
                        xts.append(x_te)

                    ps_q = pq.tile([128, 512], F32, name=f"psq_{d}_{sc}", tag="pq")
                    for e in range(EBLKS):
                        nc.tensor.matmul(
                            ps_q, lhsT=wq_t[e], rhs=xts[e],
                            start=(e == 0), stop=(e == EBLKS - 1),
                        )
                    nc.vector.tensor_scalar_add(qT[:, cols], ps_q, bq_sb[:, d : d + 1])

                    ps_k = pq.tile([128, 512], F32, name=f"psk_{d}_{sc}", tag="pq")
                    for e in range(EBLKS):
                        nc.tensor.matmul(
                            ps_k, lhsT=wk_t[e], rhs=xts[e],
                            start=(e == 0), stop=(e == EBLKS - 1),
                        )
                    nc.vector.tensor_scalar_add(kT[:, cols], ps_k, bk_sb[:, d : d + 1])

                    if side == 0:
                        # v for this s-chunk: both d blocks of the pair at once
                        for stl in range(4):
                            st = sc * 4 + stl
                            scols = slice(stl * 128, (stl + 1) * 128)
                            ps_v = pq.tile(
                                [128, 256], F32, name=f"psv_{d}_{st}", tag="pq"
                            )
                            for e in range(EBLKS):
                                nc.tensor.matmul(
                                    ps_v, lhsT=xts[e][:, scols], rhs=wv_t[e],
                                    start=(e == 0), stop=(e == EBLKS - 1),
                                )
                            va = v_pool.tile(
                                [128, 4, 65], F32R, name=f"vaug_{pair}_{st}", tag="vaug"
                            )
                            nc.sync.dma_start(
                                out=va[:, :, 64:65],
                                in_=onescol.rearrange("p (j o) -> p j o", o=1),
                            )
                            nc.vector.tensor_tensor(
                                out=va[:, :, 0:64],
                                in0=ps_v.rearrange("p (j c) -> p j c", c=64),
                                in1=bvb_sb[:, pair, :].rearrange(
                                    "p (j c) -> p j c", c=64
                                ),
                                op=ALU.add,
                            )
                            vaug_pairs[pair][st] = va

                # ---- attention for this d block (2 heads) ----
                oT_d = ot_pool.tile([128, SEQ], F32R, name=f"oT_{d}", tag="ot")
                oT.append(oT_d)
                for qc in range(SCH):
                    qcols = slice(qc * 512, (qc + 1) * 512)
                    nkt = 4 * qc + 4
                    o_ps = []
                    for h in (0, 1):
                        o_ph = pot.tile(
                            [128, 512], F32, name=f"ot_ps_{d}_{qc}_{h}", tag="pot"
                        )
                        o_ps.append(o_ph)
                    for kt in range(nkt):
                        kcols = slice(kt * 128, (kt + 1) * 128)
                        for h in (0, 1):
                            base = 64 * h
                            dsl = slice(base, base + 64)
                            s_ps = psc.tile(
                                [128, 512], F32, name=f"sc_{d}_{qc}_{kt}_{h}", tag="sc"
                            )
                            diag = kt >= 4 * qc
                            nc.tensor.matmul(
                                s_ps,
                                lhsT=kT[dsl, kcols],
                                rhs=qT[dsl, qcols],
                                start=True,
                                stop=not diag,
                            )
                            if diag:
                                nc.tensor.matmul(
                                    s_ps,
                                    lhsT=id_sb,
                                    rhs=m_sb[kt - 4 * qc],
                                    start=False,
                                    stop=True,
                                )
                            at = att_pool.tile(
                                [128, 512], F32R, name=f"at_{d}_{qc}_{kt}_{h}", tag="att"
                            )
                            nc.scalar.activation(at, s_ps, AF.Exp, scale=SCALE)
                            slot = 2 * side + h
                            nc.tensor.matmul(
                                o_ps[h][0:65, :],
                                lhsT=vaug_pairs[pair][kt][:, slot, :],
                                rhs=at,
                                start=(kt == 0),
                                stop=(kt == nkt - 1),
                            )
                    # normalize: out rows *= 1/rowsum; rowsum sits in row 64
                    for h in (0, 1):
                        lnr = s_pool.tile([1, 512], F32, name=f"lnr_{d}_{qc}_{h}", tag="lnr")
                        nc.scalar.activation(lnr, o_ps[h][64:65, :], AF.Ln)
                        sv = s_pool.tile([1, 512], F32R, name=f"sv_{d}_{qc}_{h}", tag="sv")
                        nc.scalar.activation(sv, lnr, AF.Exp, scale=-1.0)
                        # broadcast s to 64 partitions via a K=1 ones matmul
                        bc_ps = psc.tile(
                            [128, 512], F32, name=f"bc_{d}_{qc}_{h}", tag="sc"
                        )
                        nc.tensor.matmul(
                            bc_ps[0:64, :], lhsT=ones_sb, rhs=sv, start=True, stop=True
                        )
                        sb = s_pool.tile([64, 512], F32, name=f"sb_{d}_{qc}_{h}", tag="sb")
                        nc.vector.tensor_copy(out=sb, in_=bc_ps[0:64, :])
                        if h == 0:
                            nc.vector.tensor_tensor(
                                out=oT_d[0:64, qcols], in0=o_ps[0][0:64, :], in1=sb,
                                op=ALU.mult,
                            )
                        else:
                            tb = att_pool.tile(
                                [64, 512], F32, name=f"tb_{d}_{qc}", tag="tb", bufs=2
                            )
                            nc.vector.tensor_tensor(
                                out=tb, in0=o_ps[1][0:64, :], in1=sb, op=ALU.mult
                            )
                            # move to partitions 64:128 via identity matmul
                            # (full fp32: exact passthrough)
                            sh_ps = pmisc.tile(
                                [128, 512], F32, name=f"sh_{d}_{qc}", tag="pmisc"
                            )
                            nc.tensor.matmul(
                                sh_ps[64:128, :],
                                lhsT=id64_sb,
                                rhs=tb,
                                start=True,
                                stop=True,
                            )
                            nc.vector.tensor_copy(
                                out=oT_d[64:128, qcols], in_=sh_ps[64:128, :]
                            )

        # ---- output projection: yT[e, s] = sum_d Wo[d, e] * oT[d, s] ----
        for e in range(EBLKS):
            for sc in range(SCH):
                cols = slice(sc * 512, (sc + 1) * 512)
                y_ps = pmisc.tile([128, 512], F32, name=f"y_{e}_{sc}", tag="pmisc")
                for d2 in range(DBLKS):
                    nc.tensor.matmul(
                        y_ps,
                        lhsT=wo_sb[:, d2, e, :],
                        rhs=oT[d2][:, cols],
                        start=(d2 == 0),
                        stop=(d2 == DBLKS - 1),
                    )
                y_sb = y_pool.tile([128, 512], F32, name=f"ysb_{e}_{sc}", tag="y")
                nc.vector.tensor_copy(out=y_sb, in_=y_ps)
                nc.sync.dma_start(out=out[e, :, cols], in_=y_sb)


def make_in_maps(x, mask, Wq, bq, Wk, bk, Wv, bv, Wo, bo):
    """Host-side shard prep: returns list of 8 per-core input dicts."""
    x = np.asarray(x, dtype=np.float32)
    mask = np.asarray(mask)
    m2 = mask.reshape(mask.shape[-2], mask.shape[-1]).astype(bool)

    # diagonal-region additive mask tiles, [r, k, q] (valid iff q >= 128r + k)
    msk = np.empty((4, 128, 512), dtype=np.float32)
    for rr in range(4):
        msk[rr] = np.where(m2[:512, 128 * rr : 128 * rr + 128].T, 0.0, NEG)
    iden = np.eye(128, dtype=np.float32)
    iden64_np = np.eye(64, dtype=np.float32)
    ones64_np = np.ones((1, 64), dtype=np.float32)
    onescol_np = np.ones((128, 4), dtype=np.float32)

    in_maps = []
    for c in range(NCORES):
        b, g = divmod(c, 2)
        gsl = slice(g * 512, (g + 1) * 512)
        xT = np.ascontiguousarray(x[b].T).reshape(EBLKS, 128, SEQ)
        wq_c = np.ascontiguousarray(
            np.asarray(Wq, np.float32)[:, gsl].reshape(8, 128, 4, 128).transpose(0, 2, 1, 3)
        )
        wk_c = np.ascontiguousarray(
            np.asarray(Wk, np.float32)[:, gsl].reshape(8, 128, 4, 128).transpose(0, 2, 1, 3)
        )
        wv_c = np.ascontiguousarray(
            np.asarray(Wv, np.float32)[:, gsl].reshape(8, 128, 2, 256).transpose(0, 2, 1, 3)
        )
        wo_c = np.ascontiguousarray(
            np.asarray(Wo, np.float32)[gsl, :].reshape(4, 128, 8, 128).transpose(0, 2, 1, 3)
        )
        bq_c = np.ascontiguousarray(np.asarray(bq, np.float32)[gsl].reshape(4, 128).T)
        bk_c = np.ascontiguousarray(np.asarray(bk, np.float32)[gsl].reshape(4, 128).T)
        bvb_c = np.ascontiguousarray(
            np.broadcast_to(
                np.asarray(bv, np.float32)[gsl].reshape(2, 1, 256), (2, 128, 256)
            )
        )
        in_maps.append(
            {
                "xT": xT,
                "wq": wq_c,
                "wk": wk_c,
                "wv": wv_c,
                "wo": wo_c,
                "bqd": bq_c,
                "bkd": bk_c,
                "bvb": bvb_c,
                "msk": msk,
                "iden": iden,
                "iden64": iden64_np,
                "ones64": ones64_np,
                "onescol": onescol_np,
            }
        )
    return in_maps


def _is_causal(mask):
    m2 = np.asarray(mask).reshape(np.asarray(mask).shape[-2], np.asarray(mask).shape[-1])
    s = m2.shape[0]
    return m2.shape[0] == m2.shape[1] and bool(
        np.array_equal(m2.astype(bool), np.tril(np.ones((s, s), dtype=bool)))
    )


def _numpy_fallback(x, mask, Wq, bq, Wk, bk, Wv, bv, Wo, bo):
    x = np.asarray(x, np.float32)
    b, s, _ = x.shape
    q = (x @ Wq + bq).reshape(b, s, NHEADS, HDIM)
    k = (x @ Wk + bk).reshape(b, s, NHEADS, HDIM)
    v = (x @ Wv + bv).reshape(b, s, NHEADS, HDIM)
    attn = np.einsum("bqhd,bkhd->bhqk", q, k) / np.sqrt(HDIM)
    m2 = np.asarray(mask).reshape(1, 1, s, s)
    attn = np.where(m2, attn, -1e9)
    attn = attn - attn.max(axis=-1, keepdims=True)
    attn = np.exp(attn)
    attn = attn / attn.sum(axis=-1, keepdims=True)
    o = np.einsum("bhqk,bkhd->bqhd", attn, v).reshape(b, s, EMBED)
    return (o @ Wo + bo).astype(np.float32)


_NC_CACHE = None


def kernel(x, mask, Wq, bq, Wk, bk, Wv, bv, Wo, bo):
    global _NC_CACHE, LAST_EXEC_NS
    if not _is_causal(mask) or np.asarray(x).shape != (BATCH, SEQ, EMBED):
        return _numpy_fallback(x, mask, Wq, bq, Wk, bk, Wv, bv, Wo, bo)

    if _NC_CACHE is None:
        _NC_CACHE = build_bass()
    nc = _NC_CACHE

    in_maps = make_in_maps(x, mask, Wq, bq, Wk, bk, Wv, bv, Wo, bo)
    res = bass_utils.run_bass_kernel_spmd(
        nc, in_maps, core_ids=list(range(NCORES))
    )
    LAST_EXEC_NS = res.exec_time_ns

    bo = np.asarray(bo, np.float32)
    y = np.empty((BATCH, SEQ, EMBED), dtype=np.float32)
    for b in range(BATCH):
        yT = res.results[2 * b]["out"].reshape(EMBED, SEQ) + res.results[
            2 * b + 1
        ]["out"].reshape(EMBED, SEQ)
        y[b] = yT.T + bo
    return y
